# revision 32
# baseline (speedup 1.0000x reference)
"""BERT-CRF NER on Trainium2: the FLOP-dominant emissions stage (x @ W,
99.5% of the model's arithmetic) runs on device, data-parallel over batch
across 8 NeuronCores at full PE width; bias+sigmoid and the tiny
O(B*S*L^2) CRF recursion + backtrack run on host in exact f32 from the
device logits (the staged baseline already reconstructed backpointers
host-side from device scores; this extends the same approach — host math
follows the reference's f32 op order exactly, so the only divergence is
the fp16 quantization of x/W/logits).

Device pipeline per core (16 samples, 4096 token rows): the kernel is
input-bandwidth-bound, so everything is arranged around a gapless DMA
conveyor (transfers serialize FIFO by issue-readiness across the shared
HWDGE, gpsimd's SWDGE, and the DMA engines):
  - x.T streams in six [128, 4096] fp16 chunks whose queue assignment
    (sync/scalar HWDGE + gpsimd SWDGE) makes arrival order match the
    accumulation order, the last chunk split column-wise into pieces;
  - per 128-row tile, the six contraction chunks accumulate into PSUM
    with the x.T tile as the STATIONARY operand (full 128x128 PE array,
    fp16 = 1 cycle/col); PSUM accumulation groups are 2KB-bank granular,
    so each group of 8 row tiles owns a full bank with a single
    start/stop bracket;
  - per bank, an (otherwise idle) DVE copy stages f32 PSUM -> f16 logits
    and a per-bank DMA exports them as soon as that bank's last piece
    lands, overlapping the remaining stream.

Mismatches vs the reference come only from near-tie Viterbi path elements
flipped by the fp16 quantization (measured 17/32768, ~40x under the 2e-2
gate).

Shapes (hardcoded per problem spec): B=128, S=256, H=768, L=24, 8 cores.
"""

import numpy as np

B, S, H, L = 128, 256, 768, 24
N_CORES = 8
BS = B // N_CORES          # 16 samples per core
R = BS * S                 # 4096 token rows per core
NK = H // 128              # 6 contraction chunks
RT = R // 128              # 32 row tiles

_DEVICE_STATE = {}

# DMA schedule configuration (tuned against TimelineSim; transfers serialize
# FIFO by issue-readiness on the DMA engines, so queue assignment controls
# arrival order)
CFG = {
    "wk_on": "sync",
    "wk_first": False,
    "chunk_queue": [("sync", 0), ("gpsimd", 1), ("gpsimd", 3),
                    ("scalar", 2), ("scalar", 4)],
    "consume": (0, 1, 2, 3, 4),
    "pieces": [(0, 16), (16, 24), (24, 32)],
    "piece_queue": ["sync", "sync", "sync"],
    "export_queue": ["scalar", "scalar", "gpsimd", "sync"],
}


# ---------------------------------------------------------------- device ----

def _build_nc():
    import concourse.mybir as mybir
    from concourse.bass import ts
    from concourse import bacc, tile

    f32 = mybir.dt.float32
    f16 = mybir.dt.float16
    nc = bacc.Bacc()
    xT = nc.dram_tensor("xT", [H, R], f16, kind="ExternalInput")
    w = nc.dram_tensor("w", [128, NK * L], f16, kind="ExternalInput")
    # f16 logits, partition-major: lg_out[p, (rt, c)] = (x@W)[rt*128+p, c]
    # (contiguous per partition so the export is bandwidth-bound; the host
    # untangles the layout with a free transpose and applies bias+sigmoid)
    lg_out = nc.dram_tensor("lg_out", [128, RT * L], f16,
                            kind="ExternalOutput")

    with tile.TileContext(nc) as tc:
        with (
            tc.tile_pool(name="const", bufs=1) as cpool,
            tc.tile_pool(name="mm", bufs=1, space="PSUM") as mmpool,
        ):
            # W in ONE small contiguous DMA (host pre-interleaves it to
            # [128, k*L+c]). Issued after the first x chunk when wk_first is
            # False so the x stream starts on the earliest issue pipe.
            wk_all = cpool.tile([128, NK * L], f16, tag="wk_all")
            wk = [wk_all[:, k * L:(k + 1) * L] for k in range(NK)]

            def issue_wk():
                (nc.scalar if CFG["wk_on"] == "scalar"
                 else nc.sync).dma_start(out=wk_all[:, :], in_=w[:, :])
            if CFG.get("wk_first", True):
                issue_wk()

            # x chunks: DMA transfers serialize FIFO by issue-ready
            # time, with the shared HWDGE (sync+scalar interleaved) and
            # gpsimd's SWDGE pipelines determining readiness. The queue
            # assignment in CFG makes chunks land in CFG["consume"] order
            # with k5 last, split column-wise to match the exports.
            ENG = {"sync": nc.sync, "scalar": nc.scalar,
                   "gpsimd": nc.gpsimd}
            xsb = [None] * NK
            for k in range(NK):
                xsb[k] = cpool.tile([128, R], f16, name=f"x{k}", tag=f"x{k}")
            for i, (ename, k) in enumerate(CFG["chunk_queue"]):
                ENG[ename].dma_start(out=xsb[k][:, :], in_=xT[ts(k, 128), :])
                if i == 0 and not CFG.get("wk_first", True):
                    issue_wk()
            for ename, (lo, hi) in zip(CFG["piece_queue"], CFG["pieces"]):
                ENG[ename].dma_start(out=xsb[5][:, lo * 128:hi * 128],
                                     in_=xT[ts(5, 128), lo * 128:hi * 128])

            lgS = cpool.tile([128, RT * L], f16, tag="lgS")
            # PSUM accumulation groups are 2KB-bank granular (one start /
            # stop bracket per bank), so give each group of 8 row tiles its
            # own full bank: start on its first matmul (k0), stop on its
            # last (k5), everything else plain accumulate.
            SLABS = [(0, 8), (8, 16), (16, 24), (24, 32)]
            slab = [mmpool.tile([128, 512], f32, name=f"slab{j}",
                                tag=f"slab{j}") for j in range(len(SLABS))]

            def slab_of(rt):
                for sb, (lo, hi) in enumerate(SLABS):
                    if lo <= rt < hi:
                        return sb, lo, hi
                raise AssertionError

            def mm(k, rt, stop):
                sb, lo, hi = slab_of(rt)
                j = rt - lo
                nc.tensor.matmul(slab[sb][:, j * L:(j + 1) * L],
                                 xsb[k][:, ts(rt, 128)], wk[k],
                                 start=(k == 0 and rt == lo),
                                 stop=(stop and rt == hi - 1))

            # chunks k0..k4 in expected arrival order
            for k in CFG["consume"]:
                for rt in range(RT):
                    mm(k, rt, False)
            # last chunk, piece by piece: finishing matmuls -> f16
            # stage on the (otherwise idle) DVE -> export DMA. Early
            # exports ride gpsimd's SWDGE pipe so the final export's HWDGE
            # isn't queued behind them.
            pieces = CFG["pieces"]
            exq = [ENG[e] for e in CFG["export_queue"]]
            for lo_rt, hi_rt in pieces:
                for rt in range(lo_rt, hi_rt):
                    mm(NK - 1, rt, True)
                for sb, (lo, hi) in enumerate(SLABS):
                    if not (lo_rt <= lo and hi <= hi_rt):
                        continue
                    co, n = lo * L, (hi - lo) * L
                    nc.vector.tensor_copy(lgS[:, co:co + n],
                                          slab[sb][:, 0:n])
                    exq[sb].dma_start(out=lg_out[:, co:co + n],
                                      in_=lgS[:, co:co + n])
    return nc


def _run_device(xT_all, W, trace=False):
    from concourse.bass_utils import run_bass_kernel_spmd

    if "nc" not in _DEVICE_STATE:
        nc = _build_nc()
        if not nc.is_finalized():
            nc.finalize()
        _DEVICE_STATE["nc"] = nc
    nc = _DEVICE_STATE["nc"]
    w_in = np.ascontiguousarray(
        W.astype(np.float16).reshape(NK, 128, L).transpose(1, 0, 2)
        .reshape(128, NK * L))
    in_maps = [{"xT": xT_all[ci], "w": w_in} for ci in range(N_CORES)]
    res = run_bass_kernel_spmd(nc, in_maps, core_ids=list(range(N_CORES)),
                               trace=trace)
    _DEVICE_STATE["last_results"] = res
    lgs = [r["lg_out"].astype(np.float32).reshape(128, RT, L)
           .transpose(1, 0, 2).reshape(BS, S, L) for r in res.results]
    return np.concatenate(lgs, axis=0)


# ------------------------------------------------------------ host pieces ---

def _viterbi_paths(em, vmask, transitions, start_trans, end_trans):
    """Exact-f32 masked Viterbi decode, mirroring the reference. em [B,S,L]
    f32 emissions at COMPACT positions; vmask [B,S] bool."""
    lbl = np.arange(L)
    score = (start_trans[None, :] + em[:, 0]).astype(np.float32)
    bps = np.empty((S - 1, B, L), dtype=np.int64)
    T32 = transitions.astype(np.float32)
    for t in range(1, S):
        cand = score[:, :, None] + T32[None]
        best = cand.max(axis=1) + em[:, t]
        bp = cand.argmax(axis=1)
        m = vmask[:, t][:, None]
        score = np.where(m, best, score).astype(np.float32)
        bps[t - 1] = np.where(m, bp, lbl[None, :])
    final = score + end_trans[None, :]
    tag = final.argmax(axis=1)
    path = np.empty((B, S), dtype=np.int32)
    path[:, S - 1] = tag
    for t in range(S - 2, -1, -1):
        tag = np.take_along_axis(bps[t], tag[:, None], axis=1)[:, 0]
        path[:, t] = tag
    return path


def _sigmoid(x):
    out = np.empty_like(x)
    np.negative(x, out=out)
    np.exp(out, out=out)
    out += np.float32(1.0)
    np.reciprocal(out, out=out)
    return out


def _host_full(token_features, input_mask, true_label_mask, W, b,
               transitions, start_trans, end_trans):
    """General-mask fallback, mirrors the reference exactly."""
    mask = input_mask.astype(bool)
    order = np.argsort((1 - mask).astype(np.int32), axis=1, kind="stable")
    em_full = _sigmoid(
        (token_features.reshape(-1, H) @ W + b).astype(np.float32)
    ).reshape(B, S, L)
    em = np.take_along_axis(em_full, order[:, :, None], axis=1)
    em = np.concatenate([em[:, 1:], np.zeros_like(em[:, :1])], axis=1)
    n_valid = mask.sum(axis=1)
    keep = np.arange(S)[None, :] < (n_valid[:, None] - 2)
    sb = _sigmoid(np.broadcast_to(b, (L,)).astype(np.float32))
    em = np.where(keep[:, :, None], em, sb[None, None, :])
    return _viterbi_paths(em, true_label_mask != 0, transitions,
                          start_trans, end_trans)


# ------------------------------------------------------------------ entry ---

def kernel(token_features, input_mask, true_label_mask, W, b,
           transitions, start_trans, end_trans):
    token_features = np.asarray(token_features, np.float32)
    input_mask = np.asarray(input_mask)
    true_label_mask = np.asarray(true_label_mask)
    W = np.asarray(W, np.float32)
    b = np.asarray(b, np.float32)
    transitions = np.asarray(transitions, np.float32)
    start_trans = np.asarray(start_trans, np.float32)
    end_trans = np.asarray(end_trans, np.float32)

    pos = np.arange(S)[None, :]
    lengths = input_mask.sum(1)
    contig = bool(
        (input_mask == (pos < lengths[:, None])).all()
        and (true_label_mask == (pos < (lengths - 2)[:, None])).all()
        and lengths.min() >= 3
    )
    if contig:
        try:
            xh = token_features.astype(np.float16).reshape(N_CORES, R, H)
            xT_all = [np.ascontiguousarray(xh[ci].T) for ci in range(N_CORES)]
            lg = _run_device(xT_all,  W,
                             trace=_DEVICE_STATE.get("trace", False))
            emr = _sigmoid((lg + b).astype(np.float32))
            # emr [B, S, L] at ORIGINAL token positions; with contiguous
            # masks, compact position t maps to original position t+1.
            # Positions >= len-2 are masked in the recursion (score frozen,
            # backpointer identity), so their emission values are irrelevant.
            em = np.empty_like(emr)
            em[:, :S - 1] = emr[:, 1:]
            em[:, S - 1] = 0.0
            path = _viterbi_paths(em, true_label_mask != 0, transitions,
                                  start_trans, end_trans)
            _DEVICE_STATE["used"] = True
            return path
        except Exception:
            _DEVICE_STATE["used"] = False
            import traceback
            _DEVICE_STATE["error"] = traceback.format_exc()
    else:
        _DEVICE_STATE["used"] = False
        _DEVICE_STATE["error"] = "non-contiguous masks"
    return _host_full(token_features, input_mask, true_label_mask, W, b,
                      transitions, start_trans, end_trans)


# revision 34
# speedup vs baseline: 1.0004x; 1.0004x over previous
"""BERT-CRF NER on Trainium2: the FLOP-dominant emissions stage (x @ W,
99.5% of the model's arithmetic) runs on device, data-parallel over batch
across 8 NeuronCores at full PE width; bias+sigmoid and the tiny
O(B*S*L^2) CRF recursion + backtrack run on host in exact f32 from the
device logits (the staged baseline already reconstructed backpointers
host-side from device scores; this extends the same approach — host math
follows the reference's f32 op order exactly, so the only divergence is
the fp16 quantization of x/W/logits).

Device pipeline per core (16 samples, 4096 token rows): the kernel is
input-bandwidth-bound, so everything is arranged around a gapless DMA
conveyor (transfers serialize FIFO by issue-readiness across the shared
HWDGE, gpsimd's SWDGE, and the DMA engines):
  - x.T streams in six [128, 4096] fp16 chunks whose queue assignment
    (sync/scalar HWDGE + gpsimd SWDGE) makes arrival order match the
    accumulation order, the last chunk split column-wise into pieces;
  - per 128-row tile, the six contraction chunks accumulate into PSUM
    with the x.T tile as the STATIONARY operand (full 128x128 PE array,
    fp16 = 1 cycle/col); PSUM accumulation groups are 2KB-bank granular,
    so each group of 8 row tiles owns a full bank with a single
    start/stop bracket;
  - per bank, an (otherwise idle) DVE copy stages f32 PSUM -> f16 logits
    and a per-bank DMA exports them as soon as that bank's last piece
    lands, overlapping the remaining stream.

Mismatches vs the reference come only from near-tie Viterbi path elements
flipped by the fp16 quantization (measured 17/32768, ~40x under the 2e-2
gate).

Shapes (hardcoded per problem spec): B=128, S=256, H=768, L=24, 8 cores.
"""

import numpy as np

B, S, H, L = 128, 256, 768, 24
N_CORES = 8
BS = B // N_CORES          # 16 samples per core
R = BS * S                 # 4096 token rows per core
NK = H // 128              # 6 contraction chunks
RT = R // 128              # 32 row tiles

_DEVICE_STATE = {}

# DMA schedule configuration (tuned against TimelineSim; transfers serialize
# FIFO by issue-readiness on the DMA engines, so queue assignment controls
# arrival order)
CFG = {
    "wk_on": "sync",
    "wk_first": False,
    "chunk_queue": [("sync", 0), ("gpsimd", 1), ("gpsimd", 3),
                    ("scalar", 2), ("scalar", 4)],
    "consume": (0, 1, 2, 3, 4),
    "pieces": [(0, 16), (16, 24), (24, 28), (28, 32)],
    "piece_queue": ["sync", "sync", "sync", "sync"],
    "export_queue": ["scalar", "scalar", "gpsimd", "sync"],
}


# ---------------------------------------------------------------- device ----

def _build_nc():
    import concourse.mybir as mybir
    from concourse.bass import ts
    from concourse import bacc, tile

    f32 = mybir.dt.float32
    f16 = mybir.dt.float16
    nc = bacc.Bacc()
    xT = nc.dram_tensor("xT", [H, R], f16, kind="ExternalInput")
    w = nc.dram_tensor("w", [128, NK * L], f16, kind="ExternalInput")
    # f16 logits, partition-major: lg_out[p, (rt, c)] = (x@W)[rt*128+p, c]
    # (contiguous per partition so the export is bandwidth-bound; the host
    # untangles the layout with a free transpose and applies bias+sigmoid)
    lg_out = nc.dram_tensor("lg_out", [128, RT * L], f16,
                            kind="ExternalOutput")

    with tile.TileContext(nc) as tc:
        with (
            tc.tile_pool(name="const", bufs=1) as cpool,
            tc.tile_pool(name="mm", bufs=1, space="PSUM") as mmpool,
        ):
            # W in ONE small contiguous DMA (host pre-interleaves it to
            # [128, k*L+c]). Issued after the first x chunk when wk_first is
            # False so the x stream starts on the earliest issue pipe.
            wk_all = cpool.tile([128, NK * L], f16, tag="wk_all")
            wk = [wk_all[:, k * L:(k + 1) * L] for k in range(NK)]

            def issue_wk():
                (nc.scalar if CFG["wk_on"] == "scalar"
                 else nc.sync).dma_start(out=wk_all[:, :], in_=w[:, :])
            if CFG.get("wk_first", True):
                issue_wk()

            # x chunks: DMA transfers serialize FIFO by issue-ready
            # time, with the shared HWDGE (sync+scalar interleaved) and
            # gpsimd's SWDGE pipelines determining readiness. The queue
            # assignment in CFG makes chunks land in CFG["consume"] order
            # with k5 last, split column-wise to match the exports.
            ENG = {"sync": nc.sync, "scalar": nc.scalar,
                   "gpsimd": nc.gpsimd}
            xsb = [None] * NK
            for k in range(NK):
                xsb[k] = cpool.tile([128, R], f16, name=f"x{k}", tag=f"x{k}")
            for i, (ename, k) in enumerate(CFG["chunk_queue"]):
                ENG[ename].dma_start(out=xsb[k][:, :], in_=xT[ts(k, 128), :])
                if i == 0 and not CFG.get("wk_first", True):
                    issue_wk()
            for ename, (lo, hi) in zip(CFG["piece_queue"], CFG["pieces"]):
                ENG[ename].dma_start(out=xsb[5][:, lo * 128:hi * 128],
                                     in_=xT[ts(5, 128), lo * 128:hi * 128])

            lgS = cpool.tile([128, RT * L], f16, tag="lgS")
            # PSUM accumulation groups are 2KB-bank granular (one start /
            # stop bracket per bank), so give each group of 8 row tiles its
            # own full bank: start on its first matmul (k0), stop on its
            # last (k5), everything else plain accumulate.
            SLABS = [(0, 8), (8, 16), (16, 24), (24, 32)]
            slab = [mmpool.tile([128, 512], f32, name=f"slab{j}",
                                tag=f"slab{j}") for j in range(len(SLABS))]

            def slab_of(rt):
                for sb, (lo, hi) in enumerate(SLABS):
                    if lo <= rt < hi:
                        return sb, lo, hi
                raise AssertionError

            def mm(k, rt, stop):
                sb, lo, hi = slab_of(rt)
                j = rt - lo
                nc.tensor.matmul(slab[sb][:, j * L:(j + 1) * L],
                                 xsb[k][:, ts(rt, 128)], wk[k],
                                 start=(k == 0 and rt == lo),
                                 stop=(stop and rt == hi - 1))

            # chunks k0..k4 in expected arrival order
            for k in CFG["consume"]:
                for rt in range(RT):
                    mm(k, rt, False)
            # last chunk, piece by piece: finishing matmuls -> f16
            # stage on the (otherwise idle) DVE -> export DMA. Early
            # exports ride gpsimd's SWDGE pipe so the final export's HWDGE
            # isn't queued behind them.
            pieces = CFG["pieces"]
            exq = [ENG[e] for e in CFG["export_queue"]]
            for lo_rt, hi_rt in pieces:
                for rt in range(lo_rt, hi_rt):
                    mm(NK - 1, rt, True)
                for sb, (lo, hi) in enumerate(SLABS):
                    if not (lo_rt < hi <= hi_rt):
                        continue
                    co, n = lo * L, (hi - lo) * L
                    nc.vector.tensor_copy(lgS[:, co:co + n],
                                          slab[sb][:, 0:n])
                    exq[sb].dma_start(out=lg_out[:, co:co + n],
                                      in_=lgS[:, co:co + n])
    return nc


def _run_device(xT_all, W, trace=False):
    from concourse.bass_utils import run_bass_kernel_spmd

    if "nc" not in _DEVICE_STATE:
        nc = _build_nc()
        if not nc.is_finalized():
            nc.finalize()
        _DEVICE_STATE["nc"] = nc
    nc = _DEVICE_STATE["nc"]
    w_in = np.ascontiguousarray(
        W.astype(np.float16).reshape(NK, 128, L).transpose(1, 0, 2)
        .reshape(128, NK * L))
    in_maps = [{"xT": xT_all[ci], "w": w_in} for ci in range(N_CORES)]
    res = run_bass_kernel_spmd(nc, in_maps, core_ids=list(range(N_CORES)),
                               trace=trace)
    _DEVICE_STATE["last_results"] = res
    lgs = [r["lg_out"].astype(np.float32).reshape(128, RT, L)
           .transpose(1, 0, 2).reshape(BS, S, L) for r in res.results]
    return np.concatenate(lgs, axis=0)


# ------------------------------------------------------------ host pieces ---

def _viterbi_paths(em, vmask, transitions, start_trans, end_trans):
    """Exact-f32 masked Viterbi decode, mirroring the reference. em [B,S,L]
    f32 emissions at COMPACT positions; vmask [B,S] bool."""
    lbl = np.arange(L)
    score = (start_trans[None, :] + em[:, 0]).astype(np.float32)
    bps = np.empty((S - 1, B, L), dtype=np.int64)
    T32 = transitions.astype(np.float32)
    for t in range(1, S):
        cand = score[:, :, None] + T32[None]
        best = cand.max(axis=1) + em[:, t]
        bp = cand.argmax(axis=1)
        m = vmask[:, t][:, None]
        score = np.where(m, best, score).astype(np.float32)
        bps[t - 1] = np.where(m, bp, lbl[None, :])
    final = score + end_trans[None, :]
    tag = final.argmax(axis=1)
    path = np.empty((B, S), dtype=np.int32)
    path[:, S - 1] = tag
    for t in range(S - 2, -1, -1):
        tag = np.take_along_axis(bps[t], tag[:, None], axis=1)[:, 0]
        path[:, t] = tag
    return path


def _sigmoid(x):
    out = np.empty_like(x)
    np.negative(x, out=out)
    np.exp(out, out=out)
    out += np.float32(1.0)
    np.reciprocal(out, out=out)
    return out


def _host_full(token_features, input_mask, true_label_mask, W, b,
               transitions, start_trans, end_trans):
    """General-mask fallback, mirrors the reference exactly."""
    mask = input_mask.astype(bool)
    order = np.argsort((1 - mask).astype(np.int32), axis=1, kind="stable")
    em_full = _sigmoid(
        (token_features.reshape(-1, H) @ W + b).astype(np.float32)
    ).reshape(B, S, L)
    em = np.take_along_axis(em_full, order[:, :, None], axis=1)
    em = np.concatenate([em[:, 1:], np.zeros_like(em[:, :1])], axis=1)
    n_valid = mask.sum(axis=1)
    keep = np.arange(S)[None, :] < (n_valid[:, None] - 2)
    sb = _sigmoid(np.broadcast_to(b, (L,)).astype(np.float32))
    em = np.where(keep[:, :, None], em, sb[None, None, :])
    return _viterbi_paths(em, true_label_mask != 0, transitions,
                          start_trans, end_trans)


# ------------------------------------------------------------------ entry ---

def kernel(token_features, input_mask, true_label_mask, W, b,
           transitions, start_trans, end_trans):
    token_features = np.asarray(token_features, np.float32)
    input_mask = np.asarray(input_mask)
    true_label_mask = np.asarray(true_label_mask)
    W = np.asarray(W, np.float32)
    b = np.asarray(b, np.float32)
    transitions = np.asarray(transitions, np.float32)
    start_trans = np.asarray(start_trans, np.float32)
    end_trans = np.asarray(end_trans, np.float32)

    pos = np.arange(S)[None, :]
    lengths = input_mask.sum(1)
    contig = bool(
        (input_mask == (pos < lengths[:, None])).all()
        and (true_label_mask == (pos < (lengths - 2)[:, None])).all()
        and lengths.min() >= 3
    )
    if contig:
        try:
            xh = token_features.astype(np.float16).reshape(N_CORES, R, H)
            xT_all = [np.ascontiguousarray(xh[ci].T) for ci in range(N_CORES)]
            lg = _run_device(xT_all,  W,
                             trace=_DEVICE_STATE.get("trace", False))
            emr = _sigmoid((lg + b).astype(np.float32))
            # emr [B, S, L] at ORIGINAL token positions; with contiguous
            # masks, compact position t maps to original position t+1.
            # Positions >= len-2 are masked in the recursion (score frozen,
            # backpointer identity), so their emission values are irrelevant.
            em = np.empty_like(emr)
            em[:, :S - 1] = emr[:, 1:]
            em[:, S - 1] = 0.0
            path = _viterbi_paths(em, true_label_mask != 0, transitions,
                                  start_trans, end_trans)
            _DEVICE_STATE["used"] = True
            return path
        except Exception:
            _DEVICE_STATE["used"] = False
            import traceback
            _DEVICE_STATE["error"] = traceback.format_exc()
    else:
        _DEVICE_STATE["used"] = False
        _DEVICE_STATE["error"] = "non-contiguous masks"
    return _host_full(token_features, input_mask, true_label_mask, W, b,
                      transitions, start_trans, end_trans)


# revision 36
# speedup vs baseline: 1.5072x; 1.5067x over previous
"""BERT-CRF NER on Trainium2: the FLOP-dominant emissions stage (x @ W,
99.5% of the model's arithmetic) runs on device, data-parallel over batch
across 8 NeuronCores at full PE width; bias+sigmoid and the tiny
O(B*S*L^2) CRF recursion + backtrack run on host in exact f32 from the
device logits (the staged baseline already reconstructed backpointers
host-side from device scores; this extends the same approach — host math
follows the reference's f32 op order exactly, so the only divergence is
the fp16 quantization of x/W/logits).

Device pipeline per core (16 samples, 4096 token rows): the kernel is
input-bandwidth-bound, so everything is arranged around a gapless DMA
conveyor (transfers serialize FIFO by issue-readiness across the shared
HWDGE, gpsimd's SWDGE, and the DMA engines):
  - x.T streams in six [128, 4096] fp16 chunks whose queue assignment
    (sync/scalar HWDGE + gpsimd SWDGE) makes arrival order match the
    accumulation order, the last chunk split column-wise into pieces;
  - per 128-row tile, the six contraction chunks accumulate into PSUM
    with the x.T tile as the STATIONARY operand (full 128x128 PE array,
    fp16 = 1 cycle/col); PSUM accumulation groups are 2KB-bank granular,
    so each group of 8 row tiles owns a full bank with a single
    start/stop bracket;
  - per bank, an (otherwise idle) DVE copy stages f32 PSUM -> f16 logits
    and a per-bank DMA exports them as soon as that bank's last piece
    lands, overlapping the remaining stream.

Mismatches vs the reference come only from near-tie Viterbi path elements
flipped by the fp16 quantization (measured 17/32768, ~40x under the 2e-2
gate).

Shapes (hardcoded per problem spec): B=128, S=256, H=768, L=24, 8 cores.
"""

import numpy as np

B, S, H, L = 128, 256, 768, 24
N_CORES = 8
BS = B // N_CORES          # 16 samples per core
R = BS * S                 # 4096 token rows per core
NK = H // 128              # 6 contraction chunks
RT = R // 128              # 32 row tiles

_DEVICE_STATE = {}

# DMA schedule configuration (tuned against TimelineSim; transfers serialize
# FIFO by issue-readiness on the DMA engines, so queue assignment controls
# arrival order)
CFG = {
    "wk_on": "sync",
    "wk_first": False,
    "chunk_queue": [("sync", 0), ("gpsimd", 1), ("gpsimd", 3),
                    ("scalar", 2), ("scalar", 4)],
    "consume": (0, 1, 2, 3, 4),
    "pieces": [(0, 16), (16, 24), (24, 28), (28, 32)],
    "piece_queue": ["sync", "sync", "sync", "sync"],
    "export_queue": ["scalar", "scalar", "gpsimd", "sync"],
}


# ---------------------------------------------------------------- device ----

def _build_nc(rt_pad):
    import concourse.mybir as mybir
    from concourse.bass import ts
    from concourse import bacc, tile

    R_pad = rt_pad * 128
    f32 = mybir.dt.float32
    f16 = mybir.dt.float16
    nc = bacc.Bacc()
    xT = nc.dram_tensor("xT", [H, R_pad], f16, kind="ExternalInput")
    w = nc.dram_tensor("w", [128, NK * L], f16, kind="ExternalInput")
    # f16 logits, partition-major: lg_out[p, (rt, c)] = (x@W)[rt*128+p, c]
    # (contiguous per partition so the export is bandwidth-bound; the host
    # untangles the layout with a free transpose and applies bias+sigmoid)
    lg_out = nc.dram_tensor("lg_out", [128, rt_pad * L], f16,
                            kind="ExternalOutput")

    with tile.TileContext(nc) as tc:
        with (
            tc.tile_pool(name="const", bufs=1) as cpool,
            tc.tile_pool(name="mm", bufs=1, space="PSUM") as mmpool,
        ):
            # W in ONE small contiguous DMA (host pre-interleaves it to
            # [128, k*L+c]). Issued after the first x chunk when wk_first is
            # False so the x stream starts on the earliest issue pipe.
            wk_all = cpool.tile([128, NK * L], f16, tag="wk_all")
            wk = [wk_all[:, k * L:(k + 1) * L] for k in range(NK)]

            def issue_wk():
                (nc.scalar if CFG["wk_on"] == "scalar"
                 else nc.sync).dma_start(out=wk_all[:, :], in_=w[:, :])
            if CFG.get("wk_first", True):
                issue_wk()

            # x chunks: DMA transfers serialize FIFO by issue-ready
            # time, with the shared HWDGE (sync+scalar interleaved) and
            # gpsimd's SWDGE pipelines determining readiness. The queue
            # assignment in CFG makes chunks land in CFG["consume"] order
            # with k5 last, split column-wise to match the exports.
            ENG = {"sync": nc.sync, "scalar": nc.scalar,
                   "gpsimd": nc.gpsimd}
            xsb = [None] * NK
            for k in range(NK):
                xsb[k] = cpool.tile([128, R_pad], f16,
                                    name=f"x{k}", tag=f"x{k}")
            for i, (ename, k) in enumerate(CFG["chunk_queue"]):
                ENG[ename].dma_start(out=xsb[k][:, :], in_=xT[ts(k, 128), :])
                if i == 0 and not CFG.get("wk_first", True):
                    issue_wk()
            # slabs: 4 PSUM banks, rt_pad/4 row tiles each; the last
            # chunk streams in pieces matching slab boundaries (finer at
            # the end so the trailing export pipe starts ASAP)
            SG = rt_pad // 4
            SLABS = [(i * SG, (i + 1) * SG) for i in range(4)]
            pieces = [(0, 2 * SG), (2 * SG, 3 * SG),
                      (3 * SG, 3 * SG + SG // 2), (3 * SG + SG // 2, 4 * SG)]
            for ename, (lo, hi) in zip(CFG["piece_queue"], pieces):
                ENG[ename].dma_start(out=xsb[5][:, lo * 128:hi * 128],
                                     in_=xT[ts(5, 128), lo * 128:hi * 128])

            lgS = cpool.tile([128, rt_pad * L], f16, tag="lgS")
            # PSUM accumulation groups are 2KB-bank granular (one start /
            # stop bracket per bank), so give each slab its own bank:
            # start on its first matmul (k0), stop on its last (k5),
            # everything else plain accumulate.
            slab = [mmpool.tile([128, 512], f32, name=f"slab{j}",
                                tag=f"slab{j}") for j in range(len(SLABS))]

            def slab_of(rt):
                for sb, (lo, hi) in enumerate(SLABS):
                    if lo <= rt < hi:
                        return sb, lo, hi
                raise AssertionError

            def mm(k, rt, stop):
                sb, lo, hi = slab_of(rt)
                j = rt - lo
                nc.tensor.matmul(slab[sb][:, j * L:(j + 1) * L],
                                 xsb[k][:, ts(rt, 128)], wk[k],
                                 start=(k == 0 and rt == lo),
                                 stop=(stop and rt == hi - 1))

            # chunks k0..k4 in expected arrival order
            for k in CFG["consume"]:
                for rt in range(rt_pad):
                    mm(k, rt, False)
            # last chunk, piece by piece: finishing matmuls -> f16
            # stage on the (otherwise idle) DVE -> export DMA. Early
            # exports ride gpsimd's SWDGE pipe so the final export's HWDGE
            # isn't queued behind them.
            exq = [ENG[e] for e in CFG["export_queue"]]
            for lo_rt, hi_rt in pieces:
                for rt in range(lo_rt, hi_rt):
                    mm(NK - 1, rt, True)
                for sb, (lo, hi) in enumerate(SLABS):
                    if not (lo_rt < hi <= hi_rt):
                        continue
                    co, n = lo * L, (hi - lo) * L
                    nc.vector.tensor_copy(lgS[:, co:co + n],
                                          slab[sb][:, 0:n])
                    exq[sb].dma_start(out=lg_out[:, co:co + n],
                                      in_=lgS[:, co:co + n])
    return nc


def _run_device(xT_all, W, rt_pad, trace=False):
    from concourse.bass_utils import run_bass_kernel_spmd

    cache = _DEVICE_STATE.setdefault("nc_by_rt", {})
    if rt_pad not in cache:
        nc = _build_nc(rt_pad)
        if not nc.is_finalized():
            nc.finalize()
        cache[rt_pad] = nc
    nc = cache[rt_pad]
    _DEVICE_STATE["nc"] = nc
    w_in = np.ascontiguousarray(
        W.astype(np.float16).reshape(NK, 128, L).transpose(1, 0, 2)
        .reshape(128, NK * L))
    in_maps = [{"xT": xT_all[ci], "w": w_in} for ci in range(N_CORES)]
    res = run_bass_kernel_spmd(nc, in_maps, core_ids=list(range(N_CORES)),
                               trace=trace)
    _DEVICE_STATE["last_results"] = res
    lgs = [r["lg_out"].reshape(128, rt_pad, L).transpose(1, 0, 2)
           .reshape(rt_pad * 128, L) for r in res.results]
    return np.concatenate(lgs, axis=0)


# ------------------------------------------------------------ host pieces ---

def _viterbi_paths(em, vmask, transitions, start_trans, end_trans):
    """Exact-f32 masked Viterbi decode, mirroring the reference. em [B,S,L]
    f32 emissions at COMPACT positions; vmask [B,S] bool."""
    lbl = np.arange(L)
    score = (start_trans[None, :] + em[:, 0]).astype(np.float32)
    bps = np.empty((S - 1, B, L), dtype=np.int64)
    T32 = transitions.astype(np.float32)
    for t in range(1, S):
        cand = score[:, :, None] + T32[None]
        best = cand.max(axis=1) + em[:, t]
        bp = cand.argmax(axis=1)
        m = vmask[:, t][:, None]
        score = np.where(m, best, score).astype(np.float32)
        bps[t - 1] = np.where(m, bp, lbl[None, :])
    final = score + end_trans[None, :]
    tag = final.argmax(axis=1)
    path = np.empty((B, S), dtype=np.int32)
    path[:, S - 1] = tag
    for t in range(S - 2, -1, -1):
        tag = np.take_along_axis(bps[t], tag[:, None], axis=1)[:, 0]
        path[:, t] = tag
    return path


def _sigmoid(x):
    out = np.empty_like(x)
    np.negative(x, out=out)
    np.exp(out, out=out)
    out += np.float32(1.0)
    np.reciprocal(out, out=out)
    return out


def _host_full(token_features, input_mask, true_label_mask, W, b,
               transitions, start_trans, end_trans):
    """General-mask fallback, mirrors the reference exactly."""
    mask = input_mask.astype(bool)
    order = np.argsort((1 - mask).astype(np.int32), axis=1, kind="stable")
    em_full = _sigmoid(
        (token_features.reshape(-1, H) @ W + b).astype(np.float32)
    ).reshape(B, S, L)
    em = np.take_along_axis(em_full, order[:, :, None], axis=1)
    em = np.concatenate([em[:, 1:], np.zeros_like(em[:, :1])], axis=1)
    n_valid = mask.sum(axis=1)
    keep = np.arange(S)[None, :] < (n_valid[:, None] - 2)
    sb = _sigmoid(np.broadcast_to(b, (L,)).astype(np.float32))
    em = np.where(keep[:, :, None], em, sb[None, None, :])
    return _viterbi_paths(em, true_label_mask != 0, transitions,
                          start_trans, end_trans)


# ------------------------------------------------------------------ entry ---

def kernel(token_features, input_mask, true_label_mask, W, b,
           transitions, start_trans, end_trans):
    token_features = np.asarray(token_features, np.float32)
    input_mask = np.asarray(input_mask)
    true_label_mask = np.asarray(true_label_mask)
    W = np.asarray(W, np.float32)
    b = np.asarray(b, np.float32)
    transitions = np.asarray(transitions, np.float32)
    start_trans = np.asarray(start_trans, np.float32)
    end_trans = np.asarray(end_trans, np.float32)

    pos = np.arange(S)[None, :]
    lengths = input_mask.sum(1)
    contig = bool(
        (input_mask == (pos < lengths[:, None])).all()
        and (true_label_mask == (pos < (lengths - 2)[:, None])).all()
        and lengths.min() >= 3
    )
    if contig:
        try:
            # Only tokens at original positions 1..len-2 (the compacted,
            # CLS/SEP-stripped sequence) can influence the output: pack
            # exactly those rows, load-balanced across the 8 cores (the
            # matmul is per-row independent, so samples may split across
            # cores). This halves the streamed bytes on average.
            nv = (lengths - 2).astype(np.int64)
            total = int(nv.sum())
            per_core = -(-total // N_CORES)
            R_pad = min(R, max(1024, -(-per_core // 1024) * 1024))
            rt_pad = R_pad // 128
            sidx = np.repeat(np.arange(B), nv)
            starts = np.cumsum(nv) - nv
            pidx0 = np.arange(total) - np.repeat(starts, nv)  # compact pos
            xpack = np.zeros((N_CORES * R_pad, H), np.float16)
            xpack[:total] = token_features.astype(np.float16)[sidx, pidx0 + 1]
            xT_all = [np.ascontiguousarray(xpack[c * R_pad:(c + 1) * R_pad].T)
                      for c in range(N_CORES)]
            lg = _run_device(xT_all, W, rt_pad,
                             trace=_DEVICE_STATE.get("trace", False))
            # em at compact positions; masked positions (t >= len-2) never
            # influence the recursion (score frozen, identity backpointer)
            em = np.zeros((B, S, L), np.float32)
            em[sidx, pidx0] = _sigmoid(lg[:total].astype(np.float32) + b)
            path = _viterbi_paths(em, true_label_mask != 0, transitions,
                                  start_trans, end_trans)
            _DEVICE_STATE["used"] = True
            return path
        except Exception:
            _DEVICE_STATE["used"] = False
            import traceback
            _DEVICE_STATE["error"] = traceback.format_exc()
    else:
        _DEVICE_STATE["used"] = False
        _DEVICE_STATE["error"] = "non-contiguous masks"
    return _host_full(token_features, input_mask, true_label_mask, W, b,
                      transitions, start_trans, end_trans)


# revision 39
# speedup vs baseline: 1.5768x; 1.0462x over previous
"""BERT-CRF NER on Trainium2: the FLOP-dominant emissions stage (x @ W,
99.5% of the model's arithmetic) runs on device, data-parallel over batch
across 8 NeuronCores at full PE width; bias+sigmoid and the tiny
O(B*S*L^2) CRF recursion + backtrack run on host in exact f32 from the
device logits (the staged baseline already reconstructed backpointers
host-side from device scores; this extends the same approach — host math
follows the reference's f32 op order exactly, so the only divergence is
the fp16 quantization of x/W/logits).

Device pipeline per core (16 samples, 4096 token rows): the kernel is
input-bandwidth-bound, so everything is arranged around a gapless DMA
conveyor (transfers serialize FIFO by issue-readiness across the shared
HWDGE, gpsimd's SWDGE, and the DMA engines):
  - x.T streams in six [128, 4096] fp16 chunks whose queue assignment
    (sync/scalar HWDGE + gpsimd SWDGE) makes arrival order match the
    accumulation order, the last chunk split column-wise into pieces;
  - per 128-row tile, the six contraction chunks accumulate into PSUM
    with the x.T tile as the STATIONARY operand (full 128x128 PE array,
    fp16 = 1 cycle/col); PSUM accumulation groups are 2KB-bank granular,
    so each group of 8 row tiles owns a full bank with a single
    start/stop bracket;
  - per bank, an (otherwise idle) DVE copy stages f32 PSUM -> f16 logits
    and a per-bank DMA exports them as soon as that bank's last piece
    lands, overlapping the remaining stream.

Mismatches vs the reference come only from near-tie Viterbi path elements
flipped by the fp16 quantization (measured 17/32768, ~40x under the 2e-2
gate).

Shapes (hardcoded per problem spec): B=128, S=256, H=768, L=24, 8 cores.
"""

import numpy as np

B, S, H, L = 128, 256, 768, 24
N_CORES = 8
BS = B // N_CORES          # 16 samples per core
R = BS * S                 # 4096 token rows per core
NK = H // 128              # 6 contraction chunks
RT = R // 128              # 32 row tiles

_DEVICE_STATE = {}

# DMA schedule configuration (tuned against TimelineSim; transfers serialize
# FIFO by issue-readiness on the DMA engines, so queue assignment controls
# arrival order)
CFG = {
    "wk_on": "sync",
    "wk_first": False,
    "chunk_queue": [("sync", 0), ("gpsimd", 1), ("gpsimd", 3),
                    ("scalar", 2), ("scalar", 4)],
    "consume": (0, 1, 2, 3, 4),
    "piece_queue": ["sync", "sync", "sync", "sync"],
    "export_queue": ["scalar", "scalar", "gpsimd", "sync"],
    # tuned slab/piece/export layouts per row-tile count (fallback: generic)
    "slab_table": {
        16: ([(0, 8), (8, 16)], [(0, 8), (8, 16)], ["scalar", "sync"]),
    },
}


# ---------------------------------------------------------------- device ----

def _build_nc(rt_pad):
    import concourse.mybir as mybir
    from concourse.bass import ts
    from concourse import bacc, tile

    R_pad = rt_pad * 128
    f32 = mybir.dt.float32
    f16 = mybir.dt.float16
    nc = bacc.Bacc()
    xT = nc.dram_tensor("xT", [H, R_pad], f16, kind="ExternalInput")
    w = nc.dram_tensor("w", [128, NK * L], f16, kind="ExternalInput")
    # f16 logits, partition-major: lg_out[p, (rt, c)] = (x@W)[rt*128+p, c]
    # (contiguous per partition so the export is bandwidth-bound; the host
    # untangles the layout with a free transpose and applies bias+sigmoid)
    lg_out = nc.dram_tensor("lg_out", [128, rt_pad * L], f16,
                            kind="ExternalOutput")

    with tile.TileContext(nc) as tc:
        with (
            tc.tile_pool(name="const", bufs=1) as cpool,
            tc.tile_pool(name="mm", bufs=1, space="PSUM") as mmpool,
        ):
            # W in ONE small contiguous DMA (host pre-interleaves it to
            # [128, k*L+c]). Issued after the first x chunk when wk_first is
            # False so the x stream starts on the earliest issue pipe.
            wk_all = cpool.tile([128, NK * L], f16, tag="wk_all")
            wk = [wk_all[:, k * L:(k + 1) * L] for k in range(NK)]

            def issue_wk():
                (nc.scalar if CFG["wk_on"] == "scalar"
                 else nc.sync).dma_start(out=wk_all[:, :], in_=w[:, :])
            if CFG.get("wk_first", True):
                issue_wk()

            # x chunks: DMA transfers serialize FIFO by issue-ready
            # time, with the shared HWDGE (sync+scalar interleaved) and
            # gpsimd's SWDGE pipelines determining readiness. The queue
            # assignment in CFG makes chunks land in CFG["consume"] order
            # with k5 last, split column-wise to match the exports.
            ENG = {"sync": nc.sync, "scalar": nc.scalar,
                   "gpsimd": nc.gpsimd}
            xsb = [None] * NK
            for k in range(NK):
                xsb[k] = cpool.tile([128, R_pad], f16,
                                    name=f"x{k}", tag=f"x{k}")
            for i, (ename, k) in enumerate(CFG["chunk_queue"]):
                ENG[ename].dma_start(out=xsb[k][:, :], in_=xT[ts(k, 128), :])
                if i == 0 and not CFG.get("wk_first", True):
                    issue_wk()
            # slabs: PSUM bank groups; the last chunk streams in
            # pieces so slab exports start as soon as their rows land
            tbl = CFG.get("slab_table", {}).get(rt_pad)
            if tbl is not None:
                SLABS, pieces, exql = tbl
            else:
                SG = rt_pad // 4
                SLABS = [(i * SG, (i + 1) * SG) for i in range(4)]
                pieces = [(0, 2 * SG), (2 * SG, 3 * SG),
                          (3 * SG, 3 * SG + SG // 2),
                          (3 * SG + SG // 2, 4 * SG)]
                exql = CFG["export_queue"]
            for ename, (lo, hi) in zip(CFG["piece_queue"], pieces):
                ENG[ename].dma_start(out=xsb[5][:, lo * 128:hi * 128],
                                     in_=xT[ts(5, 128), lo * 128:hi * 128])

            lgS = cpool.tile([128, rt_pad * L], f16, tag="lgS")
            # PSUM accumulation groups are 2KB-bank granular (one start /
            # stop bracket per bank), so give each slab its own bank:
            # start on its first matmul (k0), stop on its last (k5),
            # everything else plain accumulate.
            slab = [mmpool.tile([128, 512], f32, name=f"slab{j}",
                                tag=f"slab{j}") for j in range(len(SLABS))]

            def slab_of(rt):
                for sb, (lo, hi) in enumerate(SLABS):
                    if lo <= rt < hi:
                        return sb, lo, hi
                raise AssertionError

            def mm(k, rt, stop):
                sb, lo, hi = slab_of(rt)
                j = rt - lo
                nc.tensor.matmul(slab[sb][:, j * L:(j + 1) * L],
                                 xsb[k][:, ts(rt, 128)], wk[k],
                                 start=(k == 0 and rt == lo),
                                 stop=(stop and rt == hi - 1))

            # chunks k0..k4 in expected arrival order
            for k in CFG["consume"]:
                for rt in range(rt_pad):
                    mm(k, rt, False)
            # last chunk, piece by piece: finishing matmuls -> f16
            # stage on the (otherwise idle) DVE -> export DMA. Early
            # exports ride gpsimd's SWDGE pipe so the final export's HWDGE
            # isn't queued behind them.
            # export queue entries may be None: copy only, merged into
            # the next slab's export DMA (one bigger contiguous transfer)
            pend_lo = None
            for lo_rt, hi_rt in pieces:
                for rt in range(lo_rt, hi_rt):
                    mm(NK - 1, rt, True)
                for sb, (lo, hi) in enumerate(SLABS):
                    if not (lo_rt < hi <= hi_rt):
                        continue
                    nc.vector.tensor_copy(
                        lgS[:, lo * L:hi * L],
                        slab[sb][:, 0:(hi - lo) * L])
                    if exql[sb] is None:
                        if pend_lo is None:
                            pend_lo = lo
                        continue
                    glo = lo if pend_lo is None else pend_lo
                    pend_lo = None
                    ENG[exql[sb]].dma_start(
                        out=lg_out[:, glo * L:hi * L],
                        in_=lgS[:, glo * L:hi * L])
    return nc


def _run_device(xT_all, W, rt_pad, trace=False):
    from concourse.bass_utils import run_bass_kernel_spmd

    cache = _DEVICE_STATE.setdefault("nc_by_rt", {})
    if rt_pad not in cache:
        nc = _build_nc(rt_pad)
        if not nc.is_finalized():
            nc.finalize()
        cache[rt_pad] = nc
    nc = cache[rt_pad]
    _DEVICE_STATE["nc"] = nc
    w_in = np.ascontiguousarray(
        W.astype(np.float16).reshape(NK, 128, L).transpose(1, 0, 2)
        .reshape(128, NK * L))
    in_maps = [{"xT": xT_all[ci], "w": w_in} for ci in range(N_CORES)]
    res = run_bass_kernel_spmd(nc, in_maps, core_ids=list(range(N_CORES)),
                               trace=trace)
    _DEVICE_STATE["last_results"] = res
    lgs = [r["lg_out"].reshape(128, rt_pad, L).transpose(1, 0, 2)
           .reshape(rt_pad * 128, L) for r in res.results]
    return np.concatenate(lgs, axis=0)


# ------------------------------------------------------------ host pieces ---

def _viterbi_paths(em, vmask, transitions, start_trans, end_trans):
    """Exact-f32 masked Viterbi decode, mirroring the reference. em [B,S,L]
    f32 emissions at COMPACT positions; vmask [B,S] bool."""
    lbl = np.arange(L)
    score = (start_trans[None, :] + em[:, 0]).astype(np.float32)
    bps = np.empty((S - 1, B, L), dtype=np.int64)
    T32 = transitions.astype(np.float32)
    for t in range(1, S):
        cand = score[:, :, None] + T32[None]
        best = cand.max(axis=1) + em[:, t]
        bp = cand.argmax(axis=1)
        m = vmask[:, t][:, None]
        score = np.where(m, best, score).astype(np.float32)
        bps[t - 1] = np.where(m, bp, lbl[None, :])
    final = score + end_trans[None, :]
    tag = final.argmax(axis=1)
    path = np.empty((B, S), dtype=np.int32)
    path[:, S - 1] = tag
    for t in range(S - 2, -1, -1):
        tag = np.take_along_axis(bps[t], tag[:, None], axis=1)[:, 0]
        path[:, t] = tag
    return path


def _sigmoid(x):
    out = np.empty_like(x)
    np.negative(x, out=out)
    np.exp(out, out=out)
    out += np.float32(1.0)
    np.reciprocal(out, out=out)
    return out


def _host_full(token_features, input_mask, true_label_mask, W, b,
               transitions, start_trans, end_trans):
    """General-mask fallback, mirrors the reference exactly."""
    mask = input_mask.astype(bool)
    order = np.argsort((1 - mask).astype(np.int32), axis=1, kind="stable")
    em_full = _sigmoid(
        (token_features.reshape(-1, H) @ W + b).astype(np.float32)
    ).reshape(B, S, L)
    em = np.take_along_axis(em_full, order[:, :, None], axis=1)
    em = np.concatenate([em[:, 1:], np.zeros_like(em[:, :1])], axis=1)
    n_valid = mask.sum(axis=1)
    keep = np.arange(S)[None, :] < (n_valid[:, None] - 2)
    sb = _sigmoid(np.broadcast_to(b, (L,)).astype(np.float32))
    em = np.where(keep[:, :, None], em, sb[None, None, :])
    return _viterbi_paths(em, true_label_mask != 0, transitions,
                          start_trans, end_trans)


# ------------------------------------------------------------------ entry ---

def kernel(token_features, input_mask, true_label_mask, W, b,
           transitions, start_trans, end_trans):
    token_features = np.asarray(token_features, np.float32)
    input_mask = np.asarray(input_mask)
    true_label_mask = np.asarray(true_label_mask)
    W = np.asarray(W, np.float32)
    b = np.asarray(b, np.float32)
    transitions = np.asarray(transitions, np.float32)
    start_trans = np.asarray(start_trans, np.float32)
    end_trans = np.asarray(end_trans, np.float32)

    pos = np.arange(S)[None, :]
    lengths = input_mask.sum(1)
    contig = bool(
        (input_mask == (pos < lengths[:, None])).all()
        and (true_label_mask == (pos < (lengths - 2)[:, None])).all()
        and lengths.min() >= 3
    )
    if contig:
        try:
            # Only tokens at original positions 1..len-2 (the compacted,
            # CLS/SEP-stripped sequence) can influence the output: pack
            # exactly those rows, load-balanced across the 8 cores (the
            # matmul is per-row independent, so samples may split across
            # cores). This halves the streamed bytes on average.
            nv = (lengths - 2).astype(np.int64)
            total = int(nv.sum())
            per_core = -(-total // N_CORES)
            R_pad = min(R, max(1024, -(-per_core // 1024) * 1024))
            rt_pad = R_pad // 128
            sidx = np.repeat(np.arange(B), nv)
            starts = np.cumsum(nv) - nv
            pidx0 = np.arange(total) - np.repeat(starts, nv)  # compact pos
            xpack = np.zeros((N_CORES * R_pad, H), np.float16)
            xpack[:total] = token_features.astype(np.float16)[sidx, pidx0 + 1]
            xT_all = [np.ascontiguousarray(xpack[c * R_pad:(c + 1) * R_pad].T)
                      for c in range(N_CORES)]
            lg = _run_device(xT_all, W, rt_pad,
                             trace=_DEVICE_STATE.get("trace", False))
            # em at compact positions; masked positions (t >= len-2) never
            # influence the recursion (score frozen, identity backpointer)
            em = np.zeros((B, S, L), np.float32)
            em[sidx, pidx0] = _sigmoid(lg[:total].astype(np.float32) + b)
            path = _viterbi_paths(em, true_label_mask != 0, transitions,
                                  start_trans, end_trans)
            _DEVICE_STATE["used"] = True
            return path
        except Exception:
            _DEVICE_STATE["used"] = False
            import traceback
            _DEVICE_STATE["error"] = traceback.format_exc()
    else:
        _DEVICE_STATE["used"] = False
        _DEVICE_STATE["error"] = "non-contiguous masks"
    return _host_full(token_features, input_mask, true_label_mask, W, b,
                      transitions, start_trans, end_trans)
